# revision 20
# baseline (speedup 1.0000x reference)
"""Trainium2 Bass kernel for nn_DConv2dBlock (deformable conv block).

Pixel-major formulation (batch sharded 2 images per core across 8 cores):
  1. offset = 3x3 conv(x): PE PSUM chain of 9 shifted matmuls per chunk
     (rhs = shifted views of a zero-padded c-major image, no staging DMA).
  2. offsets permuted to pixel-major [y, (plane, x)]; triangle masks
     Lambda(dy - s) = relu(1 - |dy - s|) built by ACT; the 81 (sy, k, sx)
     mask planes m81[y, (sy,k,sx,x)] = vy * vx via 3 DVE ops per image.
  3. products in pixel-major [y, (c, x)]: for each (k,s) one DVE op
       p = m81-plane (broadcast over c via stride-0 AP) * XT-slice
     where XT[y, (dy+2, c, xhat)] holds 5 row-shifted copies of the
     x-padded image, so both shift axes are free-dim offsets and no mask
     fan-out DMA exists at all (the channel broadcast happens inside the
     DVE operand read).
  4. per (k, img): val_k = sum of 9 products; 5 adds on DVE, 3 on gpsimd.
  5. val_k dumped to DRAM (contiguous); re-read per chunk with a
     (c, y, x) gather into channel-major [(k,c), CH] tiles; PE contracts
     all 288 (k,c) rows in a 3-matmul PSUM chain per chunk.
  6. BN stats via ACT accum_out on PSUM evacuation; 2x2 maxpool inline on
     pre-BN activations (commutes with the affine since scf >= 0); 8-core
     AllReduce of (S1, S2); tiny affine+relu on pooled maxima.

The modulator branch of the reference is dead code and skipped.
conv bias cancels inside BatchNorm and is skipped.
Requires max|offset| < 1 (checked on host; falls back to a full host
computation in the measure-zero case where it does not hold).
"""

import os
import sys
import numpy as np

for _p in ("/opt/trn_rl_repo",):
    if os.path.isdir(_p) and _p not in sys.path:
        sys.path.insert(0, _p)

B, C, H, W = 16, 32, 128, 128
O = 64
NCORES = 8
BPC = B // NCORES          # images per core
NN = H * W                 # pixels per image (16384)
EPS = 1e-5
NTOT = float(B * NN)
CH = 2048                  # chunk: 16 image rows
NCH = NN // CH             # chunks per image (8)
XH = W + 4                 # padded row width for XT (132)
QW = W + 2                 # padded cols in c-major image (130)
CW = C * W                 # free size of a (c, x) plane (4096)
KGROUPS = [(0, 4), (4, 4), (8, 1)]

_CACHE = {}
_UPTO = "full"   # "deform" | "finals" | "coll" | "full"


def _build_nc(reps=1):
    import concourse.bass as bass
    import concourse.bacc as bacc
    import concourse.mybir as mybir
    from concourse import tile
    from contextlib import ExitStack

    f32 = mybir.dt.float32
    bf16 = mybir.dt.bfloat16
    AF = mybir.ActivationFunctionType
    A = mybir.AluOpType

    nc = bacc.Bacc(num_devices=NCORES)
    x_d = nc.dram_tensor("x_sh", [BPC, C, H, W], bf16, kind="ExternalInput")
    woff_d = nc.dram_tensor("woff", [9, C, 18], bf16, kind="ExternalInput")
    wd_d = [
        nc.dram_tensor("wd0", [128, O], bf16, kind="ExternalInput"),
        nc.dram_tensor("wd1", [128, O], bf16, kind="ExternalInput"),
        nc.dram_tensor("wd2", [32, O], bf16, kind="ExternalInput"),
    ]
    offb_d = nc.dram_tensor("offb", [18, 1], f32, kind="ExternalInput")
    gam_d = nc.dram_tensor("gamma", [O, 1], f32, kind="ExternalInput")
    bet_d = nc.dram_tensor("beta", [O, 1], f32, kind="ExternalInput")
    out_d = nc.dram_tensor("out", [BPC, O, H // 2, W // 2], f32,
                           kind="ExternalOutput")

    with tile.TileContext(nc) as tc, ExitStack() as ctx:
        dram = ctx.enter_context(tc.tile_pool(name="dram", bufs=1,
                                              space="DRAM"))
        OFFd = dram.tile([BPC, 18, NN], bf16)
        VTd = dram.tile([BPC, 288, NN], bf16)       # c-major val rows
        PLd = dram.tile([BPC, O, NN // 4], bf16)    # pooled maxima
        cc_in = dram.tile([O, 2], f32)
        cc_out = dram.tile([O, 2], f32)

        consts = ctx.enter_context(tc.tile_pool(name="consts", bufs=1))
        wof_sb = consts.tile([C, 9 * 18], bf16)
        nc.sync.dma_start(
            wof_sb[:],
            bass.AP(woff_d[:].tensor, 0, [[18, C], [C * 18, 9], [1, 18]]))
        wd_sb = []
        for g in range(3):
            t = consts.tile([wd_d[g].shape[0], O], bf16, tag=f"wd{g}",
                            name=f"wd{g}")
            nc.sync.dma_start(t[:], wd_d[g][:])
            wd_sb.append(t)
        offb_sb = consts.tile([18, 1], f32)
        nc.sync.dma_start(offb_sb[:], offb_d[:])
        gam_sb = consts.tile([O, 1], f32)
        nc.sync.dma_start(gam_sb[:], gam_d[:])
        bet_sb = consts.tile([O, 1], f32)
        nc.sync.dma_start(bet_sb[:], bet_d[:])
        accp = consts.tile([O, 4 * NCH], f32)
        epsb = consts.tile([O, 1], f32)
        nc.vector.memset(epsb[:], EPS)
        sbias = []
        for s in range(3):
            t = consts.tile([128, 1], f32, tag=f"sb{s}", name=f"sb{s}")
            nc.vector.memset(t[:], float(-(s - 1)))
            sbias.append(t)

        # persistent padded image; edges zeroed once, interior rewritten
        xp_pool = ctx.enter_context(tc.tile_pool(name="xp", bufs=1))
        XT = xp_pool.tile([128, 5 * C * XH], bf16)   # 5 row-shifted copies
        nc.vector.memset(XT[:], 0.0)
        # c-major conv staging: 18 rows x 130 cols, 2 slots, edges zeroed
        xs_tiles = [xp_pool.tile([C, 18 * QW], bf16, tag=f"xs{i}",
                                 name=f"xs{i}") for i in range(2)]
        for t in xs_tiles:
            nc.vector.memset(t[:], 0.0)

        psum = ctx.enter_context(tc.tile_pool(name="psum", bufs=2,
                                              space="PSUM"))

        def v3(ap):
            return ap.rearrange("p (c x) -> p c x", x=W)

        for rep in range(reps):
            with tc.tile_pool(name="offp", bufs=1) as offp, \
                 tc.tile_pool(name="mskp", bufs=1) as mskp, \
                 tc.tile_pool(name="plp", bufs=8) as plp, \
                 tc.tile_pool(name="acp", bufs=1) as acp, \
                 tc.tile_pool(name="vcp", bufs=2) as vcp, \
                 tc.tile_pool(name="ocp", bufs=1) as ocp, \
                 tc.tile_pool(name="evp", bufs=2) as evp, \
                 tc.tile_pool(name="evq", bufs=1) as evq, \
                 tc.tile_pool(name="fin", bufs=1) as fin:

                def load_images(b):
                    """XT base copy from DRAM + 4 partition-shifted
                    SBUF->SBUF copies (big contiguous runs)."""
                    base = 2 * C * XH
                    xo = XT[:, base + 2:base + 2 + (C - 1) * XH + W]
                    xov = bass.AP(xo.tensor, xo.offset,
                                  [xo.ap[0], [XH, C], [1, W]])
                    src = x_d[b]
                    sv = bass.AP(src.tensor, src.offset,
                                 [[W, H], [H * W, C], [1, W]])
                    nc.sync.dma_start(xov, sv)
                    for d in (1, 3, 0, 4):
                        dy = d - 2
                        y0 = max(0, -dy)
                        ny = H - abs(dy)
                        dst = XT[y0:y0 + ny,
                                 d * C * XH:(d + 1) * C * XH]
                        srcv = XT[y0 + dy:y0 + dy + ny,
                                  base:base + C * XH]
                        eng = (nc.sync, nc.scalar)[d % 2]
                        eng.dma_start(dst, srcv)

                def offconv(b):
                    """3x3 conv -> OFFd[b]: PSUM chain of 9 shifted mms.

                    x rows [16ci-1, 16ci+17) staged per chunk into an
                    18-row x 130-col zero-edged c-major tile."""
                    for ci in range(NCH):
                        xs = xs_tiles[ci % 2]
                        r0 = 16 * ci - 1
                        rlo = max(0, r0)
                        rhi = min(H, r0 + 18)
                        if ci == 0:
                            nc.vector.memset(xs[:, 1:1 + W], 0.0)
                        if ci == NCH - 1:
                            nc.vector.memset(
                                xs[:, 17 * QW + 1:17 * QW + 1 + W], 0.0)
                        dsto = (rlo - r0) * QW + 1
                        dst = xs[:, dsto:dsto + (rhi - rlo - 1) * QW + W]
                        dv = bass.AP(dst.tensor, dst.offset,
                                     [dst.ap[0], [QW, rhi - rlo], [1, W]])
                        eng = (nc.sync, nc.scalar)[ci % 2]
                        eng.dma_start(dv, x_d[b, :, rlo:rhi])
                        pso = psum.tile([O, CH], f32, tag="ps", name="pso")
                        for k in range(9):
                            ki, kj = divmod(k, 3)
                            base = ki * QW + kj
                            for q4 in range(CH // 512):
                                sl = xs[:, base + q4 * 4 * QW:
                                        base + q4 * 4 * QW + 3 * QW + W]
                                rhs = bass.AP(sl.tensor, sl.offset,
                                              [sl.ap[0], [QW, 4], [1, W]])
                                nc.tensor.matmul(
                                    pso[0:18, q4 * 512:(q4 + 1) * 512],
                                    wof_sb[:, k * 18:(k + 1) * 18], rhs,
                                    start=(k == 0), stop=(k == 8))
                        oc = ocp.tile([18, CH], bf16, tag="oc", name="oc")
                        nc.scalar.activation(oc[:], pso[0:18, :],
                                             AF.Identity, bias=offb_sb[:])
                        nc.scalar.dma_start(
                            OFFd[b, :, ci * CH:(ci + 1) * CH], oc[:])

                def masks(b):
                    """offT -> vy/vx -> m81[y, (sy, k, sx, x)]."""
                    offT = offp.tile([128, 18 * W], bf16, tag="offT",
                                     name="offT")
                    src = OFFd[b]
                    nc.sync.dma_start(
                        offT[:],
                        bass.AP(src.tensor, src.offset,
                                [[W, 128], [NN, 18], [1, W]]))
                    vy = mskp.tile([128, 27 * W], bf16, tag="vy", name="vy")
                    vx = mskp.tile([128, 27 * W], bf16, tag="vx", name="vx")
                    ov = offT[:]
                    for ax, vt in ((0, vy), (1, vx)):
                        dsl = bass.AP(ov.tensor, ov.offset + ax * W,
                                      [ov.ap[0], [2 * W, 9], [1, W]])
                        for s in range(3):
                            sl = vt[:, s * 9 * W:(s + 1) * 9 * W]
                            nc.scalar.activation(sl, dsl, AF.Abs,
                                                 bias=sbias[s][:])
                            nc.scalar.activation(sl, sl, AF.Relu,
                                                 bias=1.0, scale=-1.0)
                    m81 = mskp.tile([128, 81 * W], bf16, tag="m81",
                                    name="m81")
                    vyv = vy[:]
                    vxv = vx[:]
                    for sy in range(3):
                        # out [y, (9k, (3sx, x))] = vy[sy-block k] bcast sx
                        #                         * vx[(k, sx)]
                        mo = m81[:, sy * 27 * W:(sy + 1) * 27 * W]
                        mov = bass.AP(mo.tensor, mo.offset,
                                      [mo.ap[0], [3 * W, 9], [1, 3 * W]])
                        in0 = bass.AP(vyv.tensor,
                                      vyv.offset + sy * 9 * W,
                                      [vyv.ap[0], [W, 9], [0, 3], [1, W]])
                        in1 = bass.AP(vxv.tensor, vxv.offset,
                                      [vxv.ap[0], [W, 9], [9 * W, 3],
                                       [1, W]])
                        nc.vector.tensor_tensor(mov, in0, in1, A.mult)
                    return m81

                def deform_k(b, k, m81):
                    """val_k[y, (c,x)] = sum_s m81-plane * XT-slice.

                    gpsimd sums the EARLY planes (p0..p2, available while
                    DVE is still producing) and does the final join, which
                    only the leg1 DMA consumes - so the slow gpsimd chain
                    never blocks DVE."""
                    ki, kj = divmod(k, 3)
                    m81v = m81[:]
                    xtv = XT[:]
                    prods = []
                    for si in range(9):
                        sy, sx = divmod(si, 3)
                        d = ki + sy           # 0..4 row-shift version
                        dx = kj + sx          # 0..4 col offset in xhat
                        moff = ((sy * 9 + k) * 3 + sx) * W
                        min1 = bass.AP(m81v.tensor, m81v.offset + moff,
                                       [m81v.ap[0], [0, C], [1, W]])
                        xin0 = bass.AP(xtv.tensor,
                                       xtv.offset + d * C * XH + dx,
                                       [xtv.ap[0], [XH, C], [1, W]])
                        prods.append((xin0, min1))
                    # interleaved products + out-of-place add tree on a
                    # 5-slot ring; every slot's prior tenant is consumed
                    # 1-2 ops earlier, so the pipeline never stalls.
                    planes = []
                    sums = []

                    def emit_prod(i):
                        pt = plp.tile([128, CW], bf16, tag="pt",
                                      name=f"pt{i}")
                        nc.vector.tensor_tensor(v3(pt[:]), prods[i][0],
                                                prods[i][1], A.mult)
                        planes.append(pt)

                    def emit_add(a, bb):
                        st = plp.tile([128, CW], bf16, tag="pt",
                                      name="st")
                        nc.vector.tensor_add(v3(st[:]), v3(a[:]),
                                             v3(bb[:]))
                        sums.append(st)
                        return st

                    emit_prod(0)
                    emit_prod(1)
                    emit_prod(2)
                    t = emit_add(planes[0], planes[1])
                    for i in range(3, 9):
                        emit_prod(i)
                        t = emit_add(t, planes[i - 1])
                    if _UPTO == "prodonly":
                        vt = planes[8]
                    else:
                        vt = emit_add(t, planes[8])
                    dd = VTd[b]
                    dst = bass.AP(dd.tensor, dd.offset + k * C * NN,
                                  [[W, H], [NN, C], [1, W]])
                    eng = (nc.sync, nc.scalar)[k % 2]
                    eng.dma_start(dst, vt[:])

                def final_chunk(b, ci, pooled_sl):
                    # gather c-major val tiles for this chunk from VTd
                    vals = []
                    for g, (kb, ng) in enumerate(KGROUPS):
                        vtile = vcp.tile([ng * C, CH], bf16, tag=f"val{g}",
                                         name=f"val{g}")
                        src = VTd[b]
                        inap = bass.AP(
                            src.tensor,
                            src.offset + kb * C * NN + ci * CH,
                            [[NN, ng * C], [1, CH]])
                        eng = (nc.sync, nc.scalar)[(ci + g) % 2]
                        eng.dma_start(vtile[:], inap)
                        vals.append(vtile)
                    ps = psum.tile([O, CH], f32, tag="ps", name="ps")
                    for g in range(3):
                        for q4 in range(CH // 512):
                            nc.tensor.matmul(
                                ps[:, q4 * 512:(q4 + 1) * 512],
                                wd_sb[g][:],
                                vals[g][:, q4 * 512:(q4 + 1) * 512],
                                start=(g == 0), stop=(g == 2))
                    col = 2 * (NCH * b + ci)
                    scr = evp.tile([O, CH], bf16, tag="scr", name="scr")
                    nc.scalar.activation(scr[:], ps[:], AF.Identity,
                                         accum_out=accp[:, col:col + 1])
                    rv = scr[:, :].rearrange("p (h w) -> p h w", w=W)
                    pw = evq.tile([O, CH // 2], bf16, tag="pw", name="pw")
                    pwv = pw[:, :].rearrange("p (h w) -> p h w", w=W // 2)
                    nc.vector.tensor_max(pwv, rv[:, :, 0:W:2],
                                         rv[:, :, 1:W:2])
                    pw3 = pw[:, :].rearrange("p (h w) -> p h w", w=W // 2)
                    mxs = evp.tile([O, CH // 4], bf16, tag="mxs",
                                   name="mxs")
                    nc.vector.tensor_max(
                        mxs[:].rearrange("p (h w) -> p h w", w=W // 2),
                        pw3[:, 0:16:2], pw3[:, 1:16:2])
                    nc.scalar.activation(scr[:], scr[:], AF.Square,
                                         accum_out=accp[:, col + 1:col + 2])
                    nc.sync.dma_start(pooled_sl, mxs[:])

                # ---------------- main schedule ----------------
                def deform_img(b, m81):
                    for k in range(9):
                        deform_k(b, k, m81)

                def finals_img(b):
                    for ci in range(NCH):
                        final_chunk(b, ci,
                                    PLd[b, :, ci * (CH // 4):
                                        (ci + 1) * (CH // 4)])

                if _UPTO != "coll":
                    load_images(0)
                    offconv(0)
                    m81_0 = masks(0)
                    deform_img(0, m81_0)
                    load_images(1)
                    offconv(1)
                    m81_1 = masks(1)
                    if _UPTO not in ("deform", "prodonly"):
                        finals_img(0)
                    deform_img(1, m81_1)
                    if _UPTO not in ("deform", "prodonly"):
                        finals_img(1)
                if _UPTO in ("deform", "finals", "prodonly"):
                    # keep the tail structure alive without the collective
                    nc.vector.memset(accp[:, 0:32], 1.0)

                # ---- BN: combine partials, allreduce across cores ----
                if _UPTO == "coll":
                    nc.vector.memset(accp[:, 0:32], 1.0)
                s12 = fin.tile([O, 2], f32, tag="s12", name="s12")
                nc.vector.tensor_add(accp[:, 0:16], accp[:, 0:16],
                                     accp[:, 16:32])
                nc.vector.tensor_add(accp[:, 0:8], accp[:, 0:8],
                                     accp[:, 8:16])
                nc.vector.tensor_add(accp[:, 0:4], accp[:, 0:4],
                                     accp[:, 4:8])
                nc.vector.tensor_add(s12[:, :], accp[:, 0:2], accp[:, 2:4])
                nc.sync.dma_start(cc_in[:], s12[:])
                nc.gpsimd.collective_compute(
                    "AllReduce", mybir.AluOpType.add,
                    replica_groups=[list(range(NCORES))],
                    ins=[cc_in.opt()], outs=[cc_out.opt()])

                s12r = fin.tile([O, 2], f32, tag="s12r", name="s12r")
                nc.sync.dma_start(s12r[:], cc_out[:])
                mr_ = fin.tile([O, 1], f32, tag="mr_", name="mr_")
                nc.vector.tensor_scalar_mul(mr_[:], s12r[:, 0:1],
                                            1.0 / NTOT)
                ex2 = fin.tile([O, 1], f32, tag="ex2", name="ex2")
                nc.vector.tensor_scalar_mul(ex2[:], s12r[:, 1:2],
                                            1.0 / NTOT)
                msq = fin.tile([O, 1], f32, tag="msq", name="msq")
                nc.vector.tensor_mul(msq[:], mr_[:], mr_[:])
                var = fin.tile([O, 1], f32, tag="var", name="var")
                nc.vector.tensor_sub(var[:], ex2[:], msq[:])
                sd = fin.tile([O, 1], f32, tag="sd", name="sd")
                nc.scalar.activation(sd[:], var[:], AF.Sqrt, bias=epsb[:])
                inv = fin.tile([O, 1], f32, tag="inv", name="inv")
                nc.vector.reciprocal(inv[:], sd[:])
                scf = fin.tile([O, 1], f32, tag="scf", name="scf")
                nc.vector.tensor_mul(scf[:], gam_sb[:], inv[:])
                tmp2 = fin.tile([O, 1], f32, tag="tmp2", name="tmp2")
                nc.vector.tensor_mul(tmp2[:], mr_[:], scf[:])
                bif = fin.tile([O, 1], f32, tag="bif", name="bif")
                nc.vector.tensor_sub(bif[:], bet_sb[:], tmp2[:])

                # ---- affine + relu on pooled maxima + store ----
                for b in range(BPC):
                    for q in range(16):
                        sl = PLd[b, :, q * 256:(q + 1) * 256]
                        plb = fin.tile([O, 256], bf16, tag="plb",
                                       name="plb")
                        nc.sync.dma_start(plb[:], sl)
                        r1 = fin.tile([O, 256], bf16, tag="r1", name="r1")
                        nc.vector.tensor_scalar(
                            r1[:], plb[:], scf[:], bif[:],
                            op0=mybir.AluOpType.mult,
                            op1=mybir.AluOpType.add)
                        po = fin.tile([O, 256], f32, tag="po", name="po")
                        nc.vector.tensor_scalar_max(po[:], r1[:], 0.0)
                        od = out_d[b]
                        nc.sync.dma_start(
                            bass.AP(od.tensor, od.offset + q * 256,
                                    [[NN // 4, O], [1, 256]]),
                            po[:, :])
    nc.compile()
    return nc


def _prep_inputs(x, offset_w, offset_b, conv_w, gamma, beta):
    """Host-side arrangement of weights into the layouts the kernel wants."""
    import ml_dtypes
    woff = np.zeros((9, C, 18), np.float32)
    for k in range(9):
        ki, kj = divmod(k, 3)
        woff[k] = offset_w[:, :, ki, kj].T
    wds = []
    for kb, ng in KGROUPS:
        blocks = []
        for kk in range(ng):
            ki, kj = divmod(kb + kk, 3)
            blocks.append(conv_w[:, :, ki, kj].T)      # [C, O]
        wds.append(np.ascontiguousarray(
            np.concatenate(blocks, axis=0)).astype(ml_dtypes.bfloat16))
    base = dict(
        woff=np.ascontiguousarray(woff).astype(ml_dtypes.bfloat16),
        wd0=wds[0], wd1=wds[1], wd2=wds[2],
        offb=offset_b.reshape(18, 1).astype(np.float32),
        gamma=gamma.reshape(O, 1).astype(np.float32),
        beta=beta.reshape(O, 1).astype(np.float32),
    )
    in_maps = []
    for ci in range(NCORES):
        m = dict(base)
        m["x_sh"] = np.ascontiguousarray(
            x[ci * BPC:(ci + 1) * BPC]).astype(ml_dtypes.bfloat16)
        in_maps.append(m)
    return in_maps


def _host_offsets(x, offset_w, offset_b):
    """offset = conv3x3(x, offset_w) + offset_b on host (|off|<1 check)."""
    xpad = np.pad(x, ((0, 0), (0, 0), (1, 1), (1, 1)))
    win = np.lib.stride_tricks.sliding_window_view(xpad, (3, 3), axis=(2, 3))
    cols = win.transpose(0, 2, 3, 1, 4, 5).reshape(B, NN, C * 9)
    w2 = offset_w.reshape(18, C * 9)
    off = cols @ w2.T.astype(np.float32)
    return off.reshape(B, H, W, 18).transpose(0, 3, 1, 2) + \
        offset_b.reshape(1, 18, 1, 1)


def _host_reference(x, offset_w, offset_b, conv_w, conv_b, gamma, beta):
    """Full numpy fallback (used only if some |offset| >= 1)."""
    off = _host_offsets(x, offset_w, offset_b).reshape(B, 9, 2, H, W)
    ki, kj = np.meshgrid(np.arange(3), np.arange(3), indexing="ij")
    base_y = (np.arange(H)[None, :, None] - 1 +
              ki.reshape(9)[:, None, None]).astype(np.float32)
    base_x = (np.arange(W)[None, None, :] - 1 +
              kj.reshape(9)[:, None, None]).astype(np.float32)
    py = off[:, :, 0] + base_y[None]
    px = off[:, :, 1] + base_x[None]
    y0 = np.floor(py).astype(np.int64)
    x0 = np.floor(px).astype(np.int64)
    wy = py - y0
    wx = px - x0
    bidx = np.arange(B)[:, None, None, None]

    def gather(iy, ix):
        valid = (iy >= 0) & (iy < H) & (ix >= 0) & (ix < W)
        v = x[bidx, :, np.clip(iy, 0, H - 1), np.clip(ix, 0, W - 1)]
        return np.where(valid[..., None], v, 0.0)

    val = (gather(y0, x0) * ((1 - wy) * (1 - wx))[..., None]
           + gather(y0, x0 + 1) * ((1 - wy) * wx)[..., None]
           + gather(y0 + 1, x0) * (wy * (1 - wx))[..., None]
           + gather(y0 + 1, x0 + 1) * (wy * wx)[..., None])
    out = np.einsum("bkhwc,ock->bohw", val, conv_w.reshape(O, C, 9),
                    optimize=True) + conv_b[None, :, None, None]
    m = out.mean(axis=(0, 2, 3), keepdims=True)
    v = out.var(axis=(0, 2, 3), keepdims=True)
    out = (out - m) / np.sqrt(v + EPS) * gamma[None, :, None, None] + \
        beta[None, :, None, None]
    out = np.maximum(out, 0.0)
    out = out.reshape(B, O, H // 2, 2, W // 2, 2).max(axis=(3, 5))
    return out.astype(np.float32)


def _get_nc(reps=1):
    key = ("nc", reps)
    if key not in _CACHE:
        _CACHE[key] = _build_nc(reps)
    return _CACHE[key]


def _run_device(in_maps, trace=False):
    from concourse import bass_utils
    nc = _get_nc()
    return bass_utils.run_bass_kernel_spmd(
        nc, in_maps, core_ids=list(range(NCORES)), trace=trace)


def kernel(x, offset_w, offset_b, mod_w, mod_b, conv_w, conv_b, gamma, beta,
           _trace=False, _return_results=False):
    x = np.asarray(x, np.float32)
    offset_w = np.asarray(offset_w, np.float32)
    offset_b = np.asarray(offset_b, np.float32)
    conv_w = np.asarray(conv_w, np.float32)
    conv_b = np.asarray(conv_b, np.float32)
    gamma = np.asarray(gamma, np.float32)
    beta = np.asarray(beta, np.float32)

    off = _host_offsets(x, offset_w, offset_b)
    if np.max(np.abs(off)) >= 0.999999 or np.min(gamma) < 0.0:
        return _host_reference(x, offset_w, offset_b, conv_w, conv_b,
                               gamma, beta)

    in_maps = _prep_inputs(x, offset_w, offset_b, conv_w, gamma, beta)
    res = _run_device(in_maps, trace=False)
    out = np.concatenate([res.results[i]["out"] for i in range(NCORES)],
                         axis=0)
    out = np.ascontiguousarray(out).astype(np.float32)
    if _return_results:
        return out, res
    return out


# revision 22
# speedup vs baseline: 1.0364x; 1.0364x over previous
"""Trainium2 Bass kernel for nn_DConv2dBlock (deformable conv block).

Pixel-major formulation (batch sharded 2 images per core across 8 cores):
  1. offset = 3x3 conv(x): PE PSUM chain of 9 shifted matmuls per chunk
     (rhs = shifted views of a zero-padded c-major image, no staging DMA).
  2. offsets permuted to pixel-major [y, (plane, x)]; triangle masks
     Lambda(dy - s) = relu(1 - |dy - s|) built by ACT; the 81 (sy, k, sx)
     mask planes m81[y, (sy,k,sx,x)] = vy * vx via 3 DVE ops per image.
  3. products in pixel-major [y, (c, x)]: for each (k,s) one DVE op
       p = m81-plane (broadcast over c via stride-0 AP) * XT-slice
     where XT[y, (dy+2, c, xhat)] holds 5 row-shifted copies of the
     x-padded image, so both shift axes are free-dim offsets and no mask
     fan-out DMA exists at all (the channel broadcast happens inside the
     DVE operand read).
  4. per (k, img): val_k = sum of 9 products; 5 adds on DVE, 3 on gpsimd.
  5. val_k dumped to DRAM (contiguous); re-read per chunk with a
     (c, y, x) gather into channel-major [(k,c), CH] tiles; PE contracts
     all 288 (k,c) rows in a 3-matmul PSUM chain per chunk.
  6. BN stats via ACT accum_out on PSUM evacuation; 2x2 maxpool inline on
     pre-BN activations (commutes with the affine since scf >= 0); 8-core
     AllReduce of (S1, S2); tiny affine+relu on pooled maxima.

The modulator branch of the reference is dead code and skipped.
conv bias cancels inside BatchNorm and is skipped.
Requires max|offset| < 1 (checked on host; falls back to a full host
computation in the measure-zero case where it does not hold).
"""

import os
import sys
import numpy as np

for _p in ("/opt/trn_rl_repo",):
    if os.path.isdir(_p) and _p not in sys.path:
        sys.path.insert(0, _p)

B, C, H, W = 16, 32, 128, 128
O = 64
NCORES = 8
BPC = B // NCORES          # images per core
NN = H * W                 # pixels per image (16384)
EPS = 1e-5
NTOT = float(B * NN)
CH = 2048                  # chunk: 16 image rows
NCH = NN // CH             # chunks per image (8)
XH = W + 4                 # padded row width for XT (132)
QW = W + 2                 # padded cols in c-major image (130)
CW = C * W                 # free size of a (c, x) plane (4096)
KGROUPS = [(0, 4), (4, 4), (8, 1)]

_CACHE = {}
_UPTO = "full"   # "deform" | "finals" | "coll" | "full"


def _build_nc(reps=1):
    import concourse.bass as bass
    import concourse.bacc as bacc
    import concourse.mybir as mybir
    from concourse import tile
    from contextlib import ExitStack

    f32 = mybir.dt.float32
    bf16 = mybir.dt.bfloat16
    AF = mybir.ActivationFunctionType
    A = mybir.AluOpType

    nc = bacc.Bacc(num_devices=NCORES)
    x_d = nc.dram_tensor("x_sh", [BPC, C, H, W], bf16, kind="ExternalInput")
    woff_d = nc.dram_tensor("woff", [9, C, 18], bf16, kind="ExternalInput")
    wd_d = [
        nc.dram_tensor("wd0", [128, O], bf16, kind="ExternalInput"),
        nc.dram_tensor("wd1", [128, O], bf16, kind="ExternalInput"),
        nc.dram_tensor("wd2", [32, O], bf16, kind="ExternalInput"),
    ]
    offb_d = nc.dram_tensor("offb", [18, 1], f32, kind="ExternalInput")
    gam_d = nc.dram_tensor("gamma", [O, 1], f32, kind="ExternalInput")
    bet_d = nc.dram_tensor("beta", [O, 1], f32, kind="ExternalInput")
    out_d = nc.dram_tensor("out", [BPC, O, H // 2, W // 2], f32,
                           kind="ExternalOutput")

    with tile.TileContext(nc) as tc, ExitStack() as ctx:
        dram = ctx.enter_context(tc.tile_pool(name="dram", bufs=1,
                                              space="DRAM"))
        OFFd = dram.tile([BPC, 18, NN], bf16)
        VTd = dram.tile([BPC, 2, 288, NN], bf16)    # two partial banks
        PLd = dram.tile([BPC, O, NN // 4], bf16)    # pooled maxima
        cc_in = dram.tile([O, 2], f32)
        cc_out = dram.tile([O, 2], f32)

        consts = ctx.enter_context(tc.tile_pool(name="consts", bufs=1))
        wof_sb = consts.tile([C, 9 * 18], bf16)
        nc.sync.dma_start(
            wof_sb[:],
            bass.AP(woff_d[:].tensor, 0, [[18, C], [C * 18, 9], [1, 18]]))
        wd_sb = []
        for g in range(3):
            t = consts.tile([wd_d[g].shape[0], O], bf16, tag=f"wd{g}",
                            name=f"wd{g}")
            nc.sync.dma_start(t[:], wd_d[g][:])
            wd_sb.append(t)
        offb_sb = consts.tile([18, 1], f32)
        nc.sync.dma_start(offb_sb[:], offb_d[:])
        gam_sb = consts.tile([O, 1], f32)
        nc.sync.dma_start(gam_sb[:], gam_d[:])
        bet_sb = consts.tile([O, 1], f32)
        nc.sync.dma_start(bet_sb[:], bet_d[:])
        accp = consts.tile([O, 4 * NCH], f32)
        epsb = consts.tile([O, 1], f32)
        nc.vector.memset(epsb[:], EPS)
        sbias = []
        for s in range(3):
            t = consts.tile([128, 1], f32, tag=f"sb{s}", name=f"sb{s}")
            nc.vector.memset(t[:], float(-(s - 1)))
            sbias.append(t)

        # persistent padded image; edges zeroed once, interior rewritten
        xp_pool = ctx.enter_context(tc.tile_pool(name="xp", bufs=1))
        XT = xp_pool.tile([128, 5 * C * XH], bf16)   # 5 row-shifted copies
        nc.vector.memset(XT[:], 0.0)
        # c-major conv staging: 18 rows x 130 cols, 2 slots, edges zeroed
        xs_tiles = [xp_pool.tile([C, 18 * QW], bf16, tag=f"xs{i}",
                                 name=f"xs{i}") for i in range(2)]
        for t in xs_tiles:
            nc.vector.memset(t[:], 0.0)

        psum = ctx.enter_context(tc.tile_pool(name="psum", bufs=2,
                                              space="PSUM"))

        def v3(ap):
            return ap.rearrange("p (c x) -> p c x", x=W)

        for rep in range(reps):
            with tc.tile_pool(name="offp", bufs=1) as offp, \
                 tc.tile_pool(name="mskp", bufs=1) as mskp, \
                 tc.tile_pool(name="plp", bufs=8) as plp, \
                 tc.tile_pool(name="acp", bufs=1) as acp, \
                 tc.tile_pool(name="vcp", bufs=1) as vcp, \
                 tc.tile_pool(name="ocp", bufs=1) as ocp, \
                 tc.tile_pool(name="evp", bufs=2) as evp, \
                 tc.tile_pool(name="evq", bufs=1) as evq, \
                 tc.tile_pool(name="fin", bufs=1) as fin:

                def load_images(b):
                    """XT base copy from DRAM + 4 partition-shifted
                    SBUF->SBUF copies (big contiguous runs)."""
                    base = 2 * C * XH
                    xo = XT[:, base + 2:base + 2 + (C - 1) * XH + W]
                    xov = bass.AP(xo.tensor, xo.offset,
                                  [xo.ap[0], [XH, C], [1, W]])
                    src = x_d[b]
                    sv = bass.AP(src.tensor, src.offset,
                                 [[W, H], [H * W, C], [1, W]])
                    nc.sync.dma_start(xov, sv)
                    for d in (1, 3, 0, 4):
                        dy = d - 2
                        y0 = max(0, -dy)
                        ny = H - abs(dy)
                        dst = XT[y0:y0 + ny,
                                 d * C * XH:(d + 1) * C * XH]
                        srcv = XT[y0 + dy:y0 + dy + ny,
                                  base:base + C * XH]
                        eng = (nc.sync, nc.scalar)[d % 2]
                        eng.dma_start(dst, srcv)

                def offconv(b):
                    """3x3 conv -> OFFd[b]: PSUM chain of 9 shifted mms.

                    x rows [16ci-1, 16ci+17) staged per chunk into an
                    18-row x 130-col zero-edged c-major tile."""
                    for ci in range(NCH):
                        xs = xs_tiles[ci % 2]
                        r0 = 16 * ci - 1
                        rlo = max(0, r0)
                        rhi = min(H, r0 + 18)
                        if ci == 0:
                            nc.vector.memset(xs[:, 1:1 + W], 0.0)
                        if ci == NCH - 1:
                            nc.vector.memset(
                                xs[:, 17 * QW + 1:17 * QW + 1 + W], 0.0)
                        dsto = (rlo - r0) * QW + 1
                        dst = xs[:, dsto:dsto + (rhi - rlo - 1) * QW + W]
                        dv = bass.AP(dst.tensor, dst.offset,
                                     [dst.ap[0], [QW, rhi - rlo], [1, W]])
                        eng = (nc.sync, nc.scalar)[ci % 2]
                        eng.dma_start(dv, x_d[b, :, rlo:rhi])
                        pso = psum.tile([O, CH], f32, tag="ps", name="pso")
                        for k in range(9):
                            ki, kj = divmod(k, 3)
                            base = ki * QW + kj
                            for q4 in range(CH // 512):
                                sl = xs[:, base + q4 * 4 * QW:
                                        base + q4 * 4 * QW + 3 * QW + W]
                                rhs = bass.AP(sl.tensor, sl.offset,
                                              [sl.ap[0], [QW, 4], [1, W]])
                                nc.tensor.matmul(
                                    pso[0:18, q4 * 512:(q4 + 1) * 512],
                                    wof_sb[:, k * 18:(k + 1) * 18], rhs,
                                    start=(k == 0), stop=(k == 8))
                        oc = ocp.tile([18, CH], bf16, tag="oc", name="oc")
                        nc.scalar.activation(oc[:], pso[0:18, :],
                                             AF.Identity, bias=offb_sb[:])
                        nc.scalar.dma_start(
                            OFFd[b, :, ci * CH:(ci + 1) * CH], oc[:])

                def masks(b):
                    """offT -> vy/vx -> m81[y, (sy, k, sx, x)]."""
                    offT = offp.tile([128, 18 * W], bf16, tag="offT",
                                     name="offT")
                    src = OFFd[b]
                    nc.sync.dma_start(
                        offT[:],
                        bass.AP(src.tensor, src.offset,
                                [[W, 128], [NN, 18], [1, W]]))
                    vy = mskp.tile([128, 27 * W], bf16, tag="vy", name="vy")
                    vx = mskp.tile([128, 27 * W], bf16, tag="vx", name="vx")
                    ov = offT[:]
                    for ax, vt in ((0, vy), (1, vx)):
                        dsl = bass.AP(ov.tensor, ov.offset + ax * W,
                                      [ov.ap[0], [2 * W, 9], [1, W]])
                        for s in range(3):
                            sl = vt[:, s * 9 * W:(s + 1) * 9 * W]
                            nc.scalar.activation(sl, dsl, AF.Abs,
                                                 bias=sbias[s][:])
                            nc.scalar.activation(sl, sl, AF.Relu,
                                                 bias=1.0, scale=-1.0)
                    m81 = mskp.tile([128, 81 * W], bf16, tag="m81",
                                    name="m81")
                    vyv = vy[:]
                    vxv = vx[:]
                    for sy in range(3):
                        # out [y, (9k, (3sx, x))] = vy[sy-block k] bcast sx
                        #                         * vx[(k, sx)]
                        mo = m81[:, sy * 27 * W:(sy + 1) * 27 * W]
                        mov = bass.AP(mo.tensor, mo.offset,
                                      [mo.ap[0], [3 * W, 9], [1, 3 * W]])
                        in0 = bass.AP(vyv.tensor,
                                      vyv.offset + sy * 9 * W,
                                      [vyv.ap[0], [W, 9], [0, 3], [1, W]])
                        in1 = bass.AP(vxv.tensor, vxv.offset,
                                      [vxv.ap[0], [W, 9], [9 * W, 3],
                                       [1, W]])
                        nc.vector.tensor_tensor(mov, in0, in1, A.mult)
                    return m81

                def deform_k(b, k, m81):
                    """val_k[y, (c,x)] = sum_s m81-plane * XT-slice.

                    gpsimd sums the EARLY planes (p0..p2, available while
                    DVE is still producing) and does the final join, which
                    only the leg1 DMA consumes - so the slow gpsimd chain
                    never blocks DVE."""
                    ki, kj = divmod(k, 3)
                    m81v = m81[:]
                    xtv = XT[:]
                    prods = []
                    for si in range(9):
                        sy, sx = divmod(si, 3)
                        d = ki + sy           # 0..4 row-shift version
                        dx = kj + sx          # 0..4 col offset in xhat
                        moff = ((sy * 9 + k) * 3 + sx) * W
                        min1 = bass.AP(m81v.tensor, m81v.offset + moff,
                                       [m81v.ap[0], [0, C], [1, W]])
                        xin0 = bass.AP(xtv.tensor,
                                       xtv.offset + d * C * XH + dx,
                                       [xtv.ap[0], [XH, C], [1, W]])
                        prods.append((xin0, min1))
                    # interleaved products + balanced add tree on the
                    # plane ring; two partial sums (p0-4, p5-8) each go to
                    # their own VTd bank so PE absorbs the final join.
                    planes = []

                    def emit_prod(i):
                        pt = plp.tile([128, CW], bf16, tag="pt",
                                      name=f"pt{i}")
                        nc.vector.tensor_tensor(v3(pt[:]), prods[i][0],
                                                prods[i][1], A.mult)
                        planes.append(pt)

                    def emit_add(a, bb):
                        st = plp.tile([128, CW], bf16, tag="pt",
                                      name="st")
                        nc.vector.tensor_add(v3(st[:]), v3(a[:]),
                                             v3(bb[:]))
                        return st

                    def leg1(part, vt):
                        dd = VTd[b, part]
                        dst = bass.AP(dd.tensor, dd.offset + k * C * NN,
                                      [[W, H], [NN, C], [1, W]])
                        eng = (nc.sync, nc.scalar)[(k + part) % 2]
                        eng.dma_start(dst, vt[:])

                    emit_prod(0)
                    emit_prod(1)
                    emit_prod(2)
                    emit_prod(3)
                    u1 = emit_add(planes[0], planes[1])
                    emit_prod(4)
                    u2 = emit_add(planes[2], planes[3])
                    emit_prod(5)
                    uA = emit_add(u1, u2)
                    emit_prod(6)
                    vA = emit_add(uA, planes[4])
                    leg1(0, vA)
                    emit_prod(7)
                    u3 = emit_add(planes[5], planes[6])
                    emit_prod(8)
                    u4 = emit_add(planes[7], planes[8])
                    vB = emit_add(u3, u4)
                    leg1(1, vB)

                def final_chunk(b, ci, pooled_sl):
                    # gather c-major val tiles for this chunk from VTd
                    vals = []
                    for part in range(2):
                        for g, (kb, ng) in enumerate(KGROUPS):
                            vtile = vcp.tile([ng * C, CH], bf16,
                                             tag=f"val{part}{g}",
                                             name=f"val{part}{g}")
                            src = VTd[b, part]
                            inap = bass.AP(
                                src.tensor,
                                src.offset + kb * C * NN + ci * CH,
                                [[NN, ng * C], [1, CH]])
                            eng = (nc.sync, nc.scalar)[(ci + g + part) % 2]
                            eng.dma_start(vtile[:], inap)
                            vals.append((g, vtile))
                    ps = psum.tile([O, CH], f32, tag="ps", name="ps")
                    for i, (g, vtile) in enumerate(vals):
                        for q4 in range(CH // 512):
                            nc.tensor.matmul(
                                ps[:, q4 * 512:(q4 + 1) * 512],
                                wd_sb[g][:],
                                vtile[:, q4 * 512:(q4 + 1) * 512],
                                start=(i == 0), stop=(i == 5))
                    col = 2 * (NCH * b + ci)
                    scr = evp.tile([O, CH], bf16, tag="scr", name="scr")
                    nc.scalar.activation(scr[:], ps[:], AF.Identity,
                                         accum_out=accp[:, col:col + 1])
                    rv = scr[:, :].rearrange("p (h w) -> p h w", w=W)
                    pw = evq.tile([O, CH // 2], bf16, tag="pw", name="pw")
                    pwv = pw[:, :].rearrange("p (h w) -> p h w", w=W // 2)
                    nc.vector.tensor_max(pwv, rv[:, :, 0:W:2],
                                         rv[:, :, 1:W:2])
                    pw3 = pw[:, :].rearrange("p (h w) -> p h w", w=W // 2)
                    mxs = evp.tile([O, CH // 4], bf16, tag="mxs",
                                   name="mxs")
                    nc.vector.tensor_max(
                        mxs[:].rearrange("p (h w) -> p h w", w=W // 2),
                        pw3[:, 0:16:2], pw3[:, 1:16:2])
                    nc.scalar.activation(scr[:], scr[:], AF.Square,
                                         accum_out=accp[:, col + 1:col + 2])
                    nc.sync.dma_start(pooled_sl, mxs[:])

                # ---------------- main schedule ----------------
                def deform_img(b, m81):
                    for k in range(9):
                        deform_k(b, k, m81)

                def finals_img(b):
                    for ci in range(NCH):
                        final_chunk(b, ci,
                                    PLd[b, :, ci * (CH // 4):
                                        (ci + 1) * (CH // 4)])

                if _UPTO != "coll":
                    load_images(0)
                    offconv(0)
                    m81_0 = masks(0)
                    deform_img(0, m81_0)
                    load_images(1)
                    offconv(1)
                    m81_1 = masks(1)
                    if _UPTO not in ("deform", "prodonly"):
                        finals_img(0)
                    deform_img(1, m81_1)
                    if _UPTO not in ("deform", "prodonly"):
                        finals_img(1)
                if _UPTO in ("deform", "finals", "prodonly"):
                    # keep the tail structure alive without the collective
                    nc.vector.memset(accp[:, 0:32], 1.0)

                # ---- BN: combine partials, allreduce across cores ----
                if _UPTO == "coll":
                    nc.vector.memset(accp[:, 0:32], 1.0)
                s12 = fin.tile([O, 2], f32, tag="s12", name="s12")
                nc.vector.tensor_add(accp[:, 0:16], accp[:, 0:16],
                                     accp[:, 16:32])
                nc.vector.tensor_add(accp[:, 0:8], accp[:, 0:8],
                                     accp[:, 8:16])
                nc.vector.tensor_add(accp[:, 0:4], accp[:, 0:4],
                                     accp[:, 4:8])
                nc.vector.tensor_add(s12[:, :], accp[:, 0:2], accp[:, 2:4])
                nc.sync.dma_start(cc_in[:], s12[:])
                nc.gpsimd.collective_compute(
                    "AllReduce", mybir.AluOpType.add,
                    replica_groups=[list(range(NCORES))],
                    ins=[cc_in.opt()], outs=[cc_out.opt()])

                s12r = fin.tile([O, 2], f32, tag="s12r", name="s12r")
                nc.sync.dma_start(s12r[:], cc_out[:])
                mr_ = fin.tile([O, 1], f32, tag="mr_", name="mr_")
                nc.vector.tensor_scalar_mul(mr_[:], s12r[:, 0:1],
                                            1.0 / NTOT)
                ex2 = fin.tile([O, 1], f32, tag="ex2", name="ex2")
                nc.vector.tensor_scalar_mul(ex2[:], s12r[:, 1:2],
                                            1.0 / NTOT)
                msq = fin.tile([O, 1], f32, tag="msq", name="msq")
                nc.vector.tensor_mul(msq[:], mr_[:], mr_[:])
                var = fin.tile([O, 1], f32, tag="var", name="var")
                nc.vector.tensor_sub(var[:], ex2[:], msq[:])
                sd = fin.tile([O, 1], f32, tag="sd", name="sd")
                nc.scalar.activation(sd[:], var[:], AF.Sqrt, bias=epsb[:])
                inv = fin.tile([O, 1], f32, tag="inv", name="inv")
                nc.vector.reciprocal(inv[:], sd[:])
                scf = fin.tile([O, 1], f32, tag="scf", name="scf")
                nc.vector.tensor_mul(scf[:], gam_sb[:], inv[:])
                tmp2 = fin.tile([O, 1], f32, tag="tmp2", name="tmp2")
                nc.vector.tensor_mul(tmp2[:], mr_[:], scf[:])
                bif = fin.tile([O, 1], f32, tag="bif", name="bif")
                nc.vector.tensor_sub(bif[:], bet_sb[:], tmp2[:])

                # ---- affine + relu on pooled maxima + store ----
                for b in range(BPC):
                    for q in range(16):
                        sl = PLd[b, :, q * 256:(q + 1) * 256]
                        plb = fin.tile([O, 256], bf16, tag="plb",
                                       name="plb")
                        nc.sync.dma_start(plb[:], sl)
                        r1 = fin.tile([O, 256], bf16, tag="r1", name="r1")
                        nc.vector.tensor_scalar(
                            r1[:], plb[:], scf[:], bif[:],
                            op0=mybir.AluOpType.mult,
                            op1=mybir.AluOpType.add)
                        po = fin.tile([O, 256], f32, tag="po", name="po")
                        nc.vector.tensor_scalar_max(po[:], r1[:], 0.0)
                        od = out_d[b]
                        nc.sync.dma_start(
                            bass.AP(od.tensor, od.offset + q * 256,
                                    [[NN // 4, O], [1, 256]]),
                            po[:, :])
    nc.compile()
    return nc


def _prep_inputs(x, offset_w, offset_b, conv_w, gamma, beta):
    """Host-side arrangement of weights into the layouts the kernel wants."""
    import ml_dtypes
    woff = np.zeros((9, C, 18), np.float32)
    for k in range(9):
        ki, kj = divmod(k, 3)
        woff[k] = offset_w[:, :, ki, kj].T
    wds = []
    for kb, ng in KGROUPS:
        blocks = []
        for kk in range(ng):
            ki, kj = divmod(kb + kk, 3)
            blocks.append(conv_w[:, :, ki, kj].T)      # [C, O]
        wds.append(np.ascontiguousarray(
            np.concatenate(blocks, axis=0)).astype(ml_dtypes.bfloat16))
    base = dict(
        woff=np.ascontiguousarray(woff).astype(ml_dtypes.bfloat16),
        wd0=wds[0], wd1=wds[1], wd2=wds[2],
        offb=offset_b.reshape(18, 1).astype(np.float32),
        gamma=gamma.reshape(O, 1).astype(np.float32),
        beta=beta.reshape(O, 1).astype(np.float32),
    )
    in_maps = []
    for ci in range(NCORES):
        m = dict(base)
        m["x_sh"] = np.ascontiguousarray(
            x[ci * BPC:(ci + 1) * BPC]).astype(ml_dtypes.bfloat16)
        in_maps.append(m)
    return in_maps


def _host_offsets(x, offset_w, offset_b):
    """offset = conv3x3(x, offset_w) + offset_b on host (|off|<1 check)."""
    xpad = np.pad(x, ((0, 0), (0, 0), (1, 1), (1, 1)))
    win = np.lib.stride_tricks.sliding_window_view(xpad, (3, 3), axis=(2, 3))
    cols = win.transpose(0, 2, 3, 1, 4, 5).reshape(B, NN, C * 9)
    w2 = offset_w.reshape(18, C * 9)
    off = cols @ w2.T.astype(np.float32)
    return off.reshape(B, H, W, 18).transpose(0, 3, 1, 2) + \
        offset_b.reshape(1, 18, 1, 1)


def _host_reference(x, offset_w, offset_b, conv_w, conv_b, gamma, beta):
    """Full numpy fallback (used only if some |offset| >= 1)."""
    off = _host_offsets(x, offset_w, offset_b).reshape(B, 9, 2, H, W)
    ki, kj = np.meshgrid(np.arange(3), np.arange(3), indexing="ij")
    base_y = (np.arange(H)[None, :, None] - 1 +
              ki.reshape(9)[:, None, None]).astype(np.float32)
    base_x = (np.arange(W)[None, None, :] - 1 +
              kj.reshape(9)[:, None, None]).astype(np.float32)
    py = off[:, :, 0] + base_y[None]
    px = off[:, :, 1] + base_x[None]
    y0 = np.floor(py).astype(np.int64)
    x0 = np.floor(px).astype(np.int64)
    wy = py - y0
    wx = px - x0
    bidx = np.arange(B)[:, None, None, None]

    def gather(iy, ix):
        valid = (iy >= 0) & (iy < H) & (ix >= 0) & (ix < W)
        v = x[bidx, :, np.clip(iy, 0, H - 1), np.clip(ix, 0, W - 1)]
        return np.where(valid[..., None], v, 0.0)

    val = (gather(y0, x0) * ((1 - wy) * (1 - wx))[..., None]
           + gather(y0, x0 + 1) * ((1 - wy) * wx)[..., None]
           + gather(y0 + 1, x0) * (wy * (1 - wx))[..., None]
           + gather(y0 + 1, x0 + 1) * (wy * wx)[..., None])
    out = np.einsum("bkhwc,ock->bohw", val, conv_w.reshape(O, C, 9),
                    optimize=True) + conv_b[None, :, None, None]
    m = out.mean(axis=(0, 2, 3), keepdims=True)
    v = out.var(axis=(0, 2, 3), keepdims=True)
    out = (out - m) / np.sqrt(v + EPS) * gamma[None, :, None, None] + \
        beta[None, :, None, None]
    out = np.maximum(out, 0.0)
    out = out.reshape(B, O, H // 2, 2, W // 2, 2).max(axis=(3, 5))
    return out.astype(np.float32)


def _get_nc(reps=1):
    key = ("nc", reps)
    if key not in _CACHE:
        _CACHE[key] = _build_nc(reps)
    return _CACHE[key]


def _run_device(in_maps, trace=False):
    from concourse import bass_utils
    nc = _get_nc()
    return bass_utils.run_bass_kernel_spmd(
        nc, in_maps, core_ids=list(range(NCORES)), trace=trace)


def kernel(x, offset_w, offset_b, mod_w, mod_b, conv_w, conv_b, gamma, beta,
           _trace=False, _return_results=False):
    x = np.asarray(x, np.float32)
    offset_w = np.asarray(offset_w, np.float32)
    offset_b = np.asarray(offset_b, np.float32)
    conv_w = np.asarray(conv_w, np.float32)
    conv_b = np.asarray(conv_b, np.float32)
    gamma = np.asarray(gamma, np.float32)
    beta = np.asarray(beta, np.float32)

    off = _host_offsets(x, offset_w, offset_b)
    if np.max(np.abs(off)) >= 0.999999 or np.min(gamma) < 0.0:
        return _host_reference(x, offset_w, offset_b, conv_w, conv_b,
                               gamma, beta)

    in_maps = _prep_inputs(x, offset_w, offset_b, conv_w, gamma, beta)
    res = _run_device(in_maps, trace=False)
    out = np.concatenate([res.results[i]["out"] for i in range(NCORES)],
                         axis=0)
    out = np.ascontiguousarray(out).astype(np.float32)
    if _return_results:
        return out, res
    return out
